# revision 6
# baseline (speedup 1.0000x reference)
"""CRF forward (log-partition) kernel v2 for Trainium2, 8 NeuronCores.

Same chunked-scan-with-halo decomposition as v1 (C=1024 chunks of L=64
steps, warm-up halo W, B=128 chunk-columns per core), but restructured:

1. NO per-step normalization on device. The host pre-normalizes each
   unary row by its logsumexp: eu'_t = exp(u_t - lse_t), so sum_i of
   each eu' row is 1 and the state magnitude does a +-3%/step random
   walk instead of growing by e^7.4/step. Sum_t lse_t is added back on
   the host. Per-chunk the partition contribution telescopes, so only
   TWO sums per chunk are needed (entering / leaving the owned region).
   This removes the per-step max/reciprocal/ln/scale chain entirely.

2. fp8 (float8e4) DoubleRow matmuls: both E^T (moving) and the state
   (stationary) are e4m3; contraction pairs k-subtiles so each step is
   16 matmuls of [K=256(virtual) x M=128 x N=256] at 2 elem/cycle.

3. The state stays fp8 end-to-end: DVE multiplies PSUM by eu' (bf16)
   writing fp8 z [b, i]; PE transposes z (fp8, 2 PSUM tiles, start
   only on the first transpose per tile); ScalarE copies the PSUM
   tiles back to SBUF as the next stationary via a bf16 bitcast (fp8
   byte pairs never form bf16 NaN patterns since values <= 240).

Per-step engine budget (estimated): PE 16 MM ~2.0us + 8 transposes
~0.6us; DVE 4 tensor_tensor ~1.6us; ACT 2 copies ~0.5us; DMA 0.7us.
"""

import numpy as np
import ml_dtypes
from contextlib import ExitStack

T = 65536
N = 1024
NCORES = 8
B = 128           # chunk-columns per core
L = 64            # owned steps per chunk
W = 1             # warm-up halo steps (mixing ~225x/step; sim: W=1 == W=8, 3.7e-4)
S = W + L         # 72
PERCORE = T // NCORES
KU = 8            # unary rows per DMA

_BF = ml_dtypes.bfloat16
_F8 = ml_dtypes.float8_e4m3   # TRN FP8_EXP4: max +-240

_compiled = {}


def _build_bass():
    import concourse.bacc as bacc
    import concourse.tile as tile
    from concourse import mybir
    from concourse.masks import make_identity

    bf = mybir.dt.bfloat16
    f32 = mybir.dt.float32
    f8 = mybir.dt.float8e4
    i32 = mybir.dt.int32
    ALU = mybir.AluOpType
    DR = mybir.MatmulPerfMode.DoubleRow

    nc = bacc.Bacc("TRN2", name="crf_fwd2")

    EU = nc.dram_tensor("eu", [B, S, N], bf, kind="ExternalInput")
    ET8 = nc.dram_tensor("et8", [128, 8, N], f8, kind="ExternalInput")
    OUT_S = nc.dram_tensor("sums", [B, 2], f32, kind="ExternalOutput")
    OUT_Q = nc.dram_tensor("qfin", [B, N], f8, kind="ExternalOutput")

    with tile.TileContext(nc) as tc, ExitStack() as ctx:
        consts = ctx.enter_context(tc.tile_pool(name="consts", bufs=1))
        eupool = ctx.enter_context(tc.tile_pool(name="eu", bufs=3))
        zpool = ctx.enter_context(tc.tile_pool(name="z", bufs=2))
        qpool = ctx.enter_context(tc.tile_pool(name="q", bufs=2))
        ps_mm = ctx.enter_context(tc.tile_pool(name="psmm", bufs=1, space="PSUM"))
        ps_t = ctx.enter_context(tc.tile_pool(name="pst", bufs=1, space="PSUM"))

        identb = consts.tile([128, 128], bf)
        make_identity(nc, identb)
        ident8 = consts.tile([128, 128], f8)
        nc.vector.tensor_copy(ident8[:], identb[:])

        # et_sb[k, 2g+ko, i] = E[i, 256g + 128ko + k]
        # split per g-pair so the first matmul group gates on 256KB only
        et_sb = consts.tile([128, 8, N], f8)
        for g in range(4):
            nc.sync.dma_start(
                out=et_sb[:, 2 * g : 2 * g + 2, :],
                in_=ET8.ap()[:, 2 * g : 2 * g + 2, :],
            )

        c_tile = consts.tile([128, 2], f32)

        # initial state: ones  (sum over tags = 1024 per chunk)
        q_init = consts.tile([128, 8, 128], f8)
        nc.vector.memset(q_init[:], 1.0)

        qcur = q_init

        def load_eu(t0):
            kn = min(KU, S - t0)
            til = eupool.tile([128, KU, N], bf, tag="eu", name=f"eu_{t0}")
            if t0 == 0:
                # row 0 alone first: step 0's multiply gates on 256KB
                nc.sync.dma_start(out=til[:, 0:1, :], in_=EU[:, 0:1, :])
                nc.sync.dma_start(out=til[:, 1:kn, :], in_=EU[:, 1:kn, :])
            else:
                nc.sync.dma_start(out=til[:, 0:kn, :], in_=EU[:, t0 : t0 + kn, :])
            return til

        eu_t = load_eu(0)
        eu_next = load_eu(KU)

        for s in range(S):
            ku = s % KU
            if ku == 0 and s > 0:
                eu_t = eu_next
                # prefetch the tile after next so the DMA has a full
                # 4-step (~14us) head start; avoids a ~460ns PE stall at
                # every tile boundary
                eu_next = load_eu(s + KU) if s + KU < S else None

            # psum_q[b, i] = sum_j qcur[j, b] * E^T[j, i]   (fp8 DoubleRow)
            # One PSUM tile (= one bank) per i-quarter so start=True only
            # clears its own bank and DVE reads don't serialize against
            # the next quarter's matmul writes.
            psums = [
                ps_mm.tile([128, 256], f32, tag=f"a{q}", name=f"psa{q}_{s}")
                for q in range(4)
            ]
            for q in range(4):
                cols = slice(q * 256, (q + 1) * 256)
                for g in range(4):
                    nc.tensor.matmul(
                        psums[q][:],
                        qcur[:, 2 * g : 2 * g + 2, :],
                        et_sb[:, 2 * g : 2 * g + 2, cols],
                        start=(g == 0),
                        stop=(g == 3),
                        perf_mode=DR,
                    )

            # z[b, i] = psum * eu'  -> fp8 (single rounding)
            z = zpool.tile([128, N], f8, tag="z")
            for q in range(4):
                cols = slice(q * 256, (q + 1) * 256)
                nc.vector.tensor_mul(z[:, cols], psums[q][:], eu_t[:, ku, cols])

            if s == W - 1:
                nc.vector.tensor_reduce(
                    c_tile[:, 0:1], z[:], axis=mybir.AxisListType.X, op=ALU.add
                )
            if s == S - 1:
                nc.vector.tensor_reduce(
                    c_tile[:, 1:2], z[:], axis=mybir.AxisListType.X, op=ALU.add
                )
                nc.sync.dma_start(out=OUT_Q[:, :], in_=z[:])
                nc.sync.dma_start(out=OUT_S[:, :], in_=c_tile[:])
                break

            # transpose z -> qnext[i, b] (fp8), 4 blocks per PSUM tile
            qnext = qpool.tile([128, 8, 128], f8, tag="qn")
            # fp8 transpose-mode writes with output element step 2.
            # 4 transposes per PSUM tile: only the first of each group
            # pays the unpipelined ~229ns start; followers run at ~55ns.
            # (Pair-granular tiles measured slower: each start=True group
            # costs one 229ns transpose.)
            psts = [
                ps_t.tile([128, 1024], f8, tag=f"pst{p}", name=f"pst{p}_{s}")
                for p in range(2)
            ]
            for it in range(8):
                dst = psts[it // 4]
                r = it % 4
                nc.tensor.matmul(
                    dst[:, r * 256 : (r + 1) * 256 : 2],
                    z[:, it * 128 : (it + 1) * 128],
                    ident8[:],
                    is_transpose=True,
                    start=(r == 0),
                    stop=(r == 3),
                    skip_group_check=True,
                )
            # first copy on DVE (needs only T0-3, lands right after this
            # step's multiplies in DVE's FIFO — no circular wait), second
            # on ACT: halves the serial copy chain that gates the next
            # step's matmul groups. (Putting the LAST copy on DVE instead
            # couples DVE to T7 and regressed +19us.)
            nc.vector.tensor_copy(qnext[:, 0:4, :], psts[0][:, 0:1024:2])
            nc.scalar.copy(qnext[:, 4:8, :], psts[1][:, 0:1024:2])
            qcur = qnext

    nc.finalize()
    return nc


def _get_nc():
    if "nc" not in _compiled:
        _compiled["nc"] = _build_bass()
    return _compiled["nc"]


def _prep_inputs(unary, transitions, start_idx):
    """Host-side: lse-normalized exp(unary) in bf16 + halo gather; fp8 E^T."""
    unary = np.asarray(unary, dtype=np.float32)
    transitions = np.asarray(transitions, dtype=np.float32)

    m = unary.max(axis=1)
    lse = m + np.log(np.exp(unary - m[:, None]).sum(axis=1, dtype=np.float32))
    _compiled["lse_sum"] = float(lse.astype(np.float64).sum())
    eu = np.exp(unary - lse[:, None]).astype(_BF)  # rows sum to 1

    # fake warm-up rows for chunk 0: one-hot at start_idx; first row scaled
    # 1/8, later rows 1/E_ss so the state magnitude stays ~128.
    fake = np.zeros((W, N), dtype=np.float32)
    fake[0, start_idx] = 1.0 / 8.0
    fake[1:, start_idx] = np.exp(-transitions[start_idx, start_idx])
    g = np.concatenate([fake.astype(_BF), eu], axis=0)  # [W+T, N] bf16

    E = np.exp(transitions)  # [i, j], entries in ~[0.6, 1.7]
    # et8[k, 2g+ko, i] = E[i, 256g+128ko+k]
    et8 = np.ascontiguousarray(
        E.T.reshape(4, 2, 128, N).transpose(2, 0, 1, 3).reshape(128, 8, N)
    ).astype(_F8)

    row_bytes = N * 2
    in_maps = []
    for c in range(NCORES):
        base = g[c * PERCORE :]
        view = np.lib.stride_tricks.as_strided(
            base, shape=(B, S, N), strides=(L * row_bytes, row_bytes, 2)
        )
        in_maps.append({"eu": np.ascontiguousarray(view), "et8": et8})
    return in_maps


def _combine(results, transitions, end_idx):
    transitions = np.asarray(transitions, dtype=np.float32)
    s0 = np.concatenate([r["sums"][:, 0] for r in results]).astype(np.float64)
    s1 = np.concatenate([r["sums"][:, 1] for r in results]).astype(np.float64)
    total = _compiled["lse_sum"] + float(np.sum(np.log(s1) - np.log(s0)))
    q_T = results[-1]["qfin"][B - 1].astype(np.float64)
    tau = np.exp(transitions[end_idx].astype(np.float64))
    total += float(np.log(np.dot(tau, q_T))) - float(np.log(s1[-1]))
    return total


def kernel(unary, transitions, start_idx, end_idx, _trace=False):
    from concourse.bass_utils import run_bass_kernel_spmd

    start_idx = int(np.asarray(start_idx))
    end_idx = int(np.asarray(end_idx))

    nc = _get_nc()
    in_maps = _prep_inputs(unary, transitions, start_idx)
    res = run_bass_kernel_spmd(nc, in_maps, core_ids=list(range(NCORES)), trace=_trace)
    _compiled["last_result"] = res
    logZ = _combine(res.results, transitions, end_idx)
    return np.array(logZ, dtype=np.float32)
